# revision 12
# baseline (speedup 1.0000x reference)
"""Trainium2 Bass kernel for CausalSelfAttention (B=2, T=2048, D=1024, H=16).

v5: mask-compacted sequences. ~half the tokens have attention_mask == 0;
masked keys provably don't affect valid queries (softmax over -inf) and
masked-query rows output exactly b_proj. So the host compacts each batch
to its valid tokens (order preserved -> causal structure is exactly
preserved), pads per batch to a multiple of 128, and the device runs the
same Megatron-style 8-core kernel on the short sequences (~1024+1152
tokens instead of 2048+2048). Attention area shrinks ~3.2x, projections
~1.9x. Valid-token rows are scattered back on the host; masked rows get
b_proj.

Device structure per core (heads {2c, 2c+1}), inherited from v4:
  - column-parallel c_attn (384 of 3072 features), full attention for
    2 heads x 2 batches, row-parallel c_proj; host sums 8 partials.
  - gapless-PE design (HAM): warm-up matmuls during initial DMA wait,
    QK pairs into disjoint PE row groups, phase-split spans with
    injectable filler (qkv chunks / deferred PV / c_proj tiles).
  - denominator trick: v_nat carries a mask column (1 valid / 1e-18
    pad) so the softmax denominator comes out of the PV matmul as
    row 64; normalization multiplies by its reciprocal broadcast.
  - pad-token rows produce small garbage and are discarded host-side,
    so the old query-mask multiply in the c_proj eviction is gone;
    evictions alternate DVE/ACT to balance engine load.
  - spans of <=512 queries; narrower remainder spans pack G=512//qw
    k-tiles into one [128,1024] S-tile so EXP stays wide.
"""

import functools
from collections import deque

import numpy as np
import ml_dtypes

import concourse.bass as bass
import concourse.mybir as mybir
import concourse.tile as tile
from concourse import bacc
from concourse.bass_utils import run_bass_kernel_spmd
from concourse.masks import make_upper_triangular, make_identity

BF16 = mybir.dt.bfloat16
F32 = mybir.dt.float32
AF = mybir.ActivationFunctionType
OP = mybir.AluOpType

B, T, D, NH = 2, 2048, 1024, 16
DH = 64                  # head dim
HPC = 2                  # heads per core
NCORES = 8
P = 128
KC = D // P              # 8 contraction tiles for qkv
SPAN = 512               # max q-span per softmax pass
QSCALE = 1.0 / np.sqrt(DH)
ESHIFT = -10.0           # constant exp shift; cancels in softmax ratio
VW = 2 * DH + 2          # v_nat row width: [h0 v | m | h1 v | m]
GW = 512                 # qkv token-group width


def build(TP0, TP1):
    TPs = (TP0, TP1)
    offs = (0, TP0)
    TT = TP0 + TP1
    NT = TT // P

    # qkv token groups; first one small so its DMA lands just as the
    # PE warm-up stream ends
    groups = []
    t = 0
    while t < TT:
        w = min(256 if t == 0 else GW, TT - t)
        groups.append((t, w))
        t += w
    NG = len(groups)

    # attention spans (b, local q0, width)
    spans = []
    for b in (0, 1):
        q = 0
        while q < TPs[b]:
            w = min(SPAN, TPs[b] - q)
            spans.append((b, q, w))
            q += w

    nc = bacc.Bacc(None)

    xT = nc.dram_tensor("xT", [D, TT], BF16, kind="ExternalInput")
    wqkv = nc.dram_tensor("wqkv", [KC, P, 3 * P], BF16, kind="ExternalInput")
    bqkv = nc.dram_tensor("bqkv", [P, 3], F32, kind="ExternalInput")
    wproj = nc.dram_tensor("wproj", [P, D], BF16, kind="ExternalInput")
    mcolden = nc.dram_tensor("mcolden", [P, NT, 1], F32, kind="ExternalInput")
    out = nc.dram_tensor("out", [TT, D], BF16, kind="ExternalOutput")

    with tile.TileContext(nc) as tc:
        with (
            tc.tile_pool(name="singles", bufs=1) as singles,
            tc.tile_pool(name="stage", bufs=2) as stage,
            tc.tile_pool(name="pt", bufs=24) as ptp,
            tc.tile_pool(name="rows", bufs=2) as rows,
            tc.tile_pool(name="outs", bufs=3) as outs,
            tc.tile_pool(name="st", bufs=2, space="PSUM") as ps_st,
            tc.tile_pool(name="pv", bufs=1, space="PSUM") as ps_pv,
            tc.tile_pool(name="popq", bufs=2, space="PSUM") as ps_pq,
        ):
            # warm-up fodder first: a DVE memset is ready ~0.3us in
            warm_in = singles.tile([P, P], BF16)
            nc.vector.memset(warm_in, 0.5)

            eshift_sb = singles.tile([P, 1], F32)
            nc.vector.memset(eshift_sb, ESHIFT)
            ut_sb = singles.tile([P, P], BF16)  # keep q >= k
            make_upper_triangular(nc, ut_sb, val=1.0, diag=True)
            ident = singles.tile([P, P], BF16)
            make_identity(nc, ident)

            # input DMAs, most-urgent first
            wqkv_sb = singles.tile([P, KC, 3 * P], BF16)
            nc.sync.dma_start(out=wqkv_sb, in_=wqkv.rearrange("k p m -> p k m"))
            xT_sb = singles.tile([P, KC, TT], BF16)
            g0w = groups[0][1]
            for k in range(KC):
                nc.sync.dma_start(out=xT_sb[:, k, 0:g0w],
                                  in_=xT[k * P:(k + 1) * P, 0:g0w])
            bqkv_sb = singles.tile([P, 3], F32)
            nc.sync.dma_start(out=bqkv_sb, in_=bqkv[:, :])
            mden_sb = singles.tile([P, NT, 1], F32)
            nc.sync.dma_start(out=mden_sb, in_=mcolden[:, :, :])
            wproj_sb = singles.tile([P, D], BF16)
            nc.sync.dma_start(out=wproj_sb, in_=wproj[:, :])

            # PE warm-up through the DMA wait. Wide (N=512) matmuls give
            # ~full streaming duty so the HAM un-throttles after ~3.4us;
            # narrow low-duty warm-ups never trip it and the real stream
            # then starts (and stays) at half clock.
            warm_rhs = singles.tile([P, 512], BF16)
            nc.vector.memset(warm_rhs, 0.25)
            for w in range(8):
                wps = ps_pq.tile([P, 512], F32, tag="popq", name=f"warm{w}")
                nc.tensor.matmul(wps[:], warm_in[:], warm_rhs[:],
                                 start=True, stop=True)

            qT_sb = singles.tile([P, TT], BF16)   # rows: h0 d0..63 | h1 d0..63
            kT_sb = singles.tile([P, TT], BF16)
            yT_sb = singles.tile([P, TT], BF16)
            v_nat = singles.tile([P, NT, VW], BF16)

            # denominator-mask columns for all k-tiles in two bulk ops
            nc.vector.tensor_copy(out=v_nat[:, :, DH:DH + 1], in_=mden_sb)
            nc.vector.tensor_copy(out=v_nat[:, :, VW - 1:VW], in_=mden_sb)

            # ---- injectable chunk emitters ----
            def emit_qkv_chunk(g, m):
                tok0, w = groups[g]
                if m == 0 and g + 1 < NG:   # prefetch next group's xT
                    nt0, nw = groups[g + 1]
                    for k in range(KC):
                        nc.sync.dma_start(
                            out=xT_sb[:, k, nt0:nt0 + nw],
                            in_=xT[k * P:(k + 1) * P, nt0:nt0 + nw])
                pq = ps_pq.tile([P, w], F32, tag="popq", name=f"pq{g}_{m}")
                for k in range(KC):
                    nc.tensor.matmul(
                        pq[:], wqkv_sb[:, k, m * P:(m + 1) * P],
                        xT_sb[:, k, tok0:tok0 + w],
                        start=(k == 0), stop=(k == KC - 1))
                csl = slice(tok0, tok0 + w)
                if m == 0:
                    nc.vector.tensor_scalar(
                        qT_sb[:, csl], pq[:], QSCALE, bqkv_sb[:, 0:1],
                        OP.mult, OP.add)
                elif m == 1:
                    nc.vector.tensor_scalar(
                        kT_sb[:, csl], pq[:], 1.0, bqkv_sb[:, 1:2],
                        OP.mult, OP.add)
                else:
                    vst = stage.tile([P, w], BF16, tag="vst", name=f"vst{g}")
                    nc.vector.tensor_copy(out=vst[:], in_=pq[:])
                    nq = w // P
                    j0 = tok0 // P
                    vtp = ps_pq.tile([P, nq, P], BF16, tag="popq",
                                     name=f"vtp{g}")
                    for q4 in range(nq):
                        nc.tensor.transpose(
                            vtp[:, q4, :], vst[:, q4 * P:(q4 + 1) * P],
                            ident[:])
                    nc.vector.tensor_copy(
                        out=v_nat[:, j0:j0 + nq, 0:DH], in_=vtp[:, :, 0:DH])
                    nc.vector.tensor_copy(
                        out=v_nat[:, j0:j0 + nq, DH + 1:2 * DH + 1],
                        in_=vtp[:, :, DH:2 * DH])

            evict_rr = [0]

            def emit_proj_tile(tt):
                ob = outs.tile([P, D], BF16, tag="ob", name=f"ob{tt}")
                for half in range(2):
                    po = ps_pq.tile([P, 512], F32, tag="popq",
                                    name=f"po{tt}_{half}")
                    nc.tensor.matmul(
                        po[:], yT_sb[:, tt * P:(tt + 1) * P],
                        wproj_sb[:, half * 512:(half + 1) * 512],
                        start=True, stop=True)
                    osl = ob[:, half * 512:(half + 1) * 512]
                    if evict_rr[0] % 2 == 0:
                        nc.vector.tensor_copy(out=osl, in_=po[:])
                    else:
                        nc.scalar.copy(out=osl, in_=po[:])
                    evict_rr[0] += 1
                nc.sync.dma_start(out=out[tt * P:(tt + 1) * P, :], in_=ob)

            # pending injectable work, popped between attention k-tiles
            inject_q = deque()
            pending_qkv = {}

            def _pop_emit():
                tag, fn = inject_q.popleft()
                if tag is not None:
                    pending_qkv[tag] -= 1
                fn()

            def drain_inject(k):
                for _ in range(k):
                    if not inject_q:
                        return
                    _pop_emit()

            qkv_done = set()

            def require_group(g):
                if g in qkv_done:
                    while pending_qkv.get(g, 0) > 0:
                        _pop_emit()
                    return
                qkv_done.add(g)
                for m in range(3):
                    emit_qkv_chunk(g, m)

            def queue_group(g):
                if g in qkv_done:
                    return
                qkv_done.add(g)
                pending_qkv[g] = 3
                for m in range(3):
                    inject_q.append(
                        (g, functools.partial(emit_qkv_chunk, g, m)))

            holdback = []  # early proj tiles deferred to the endgame

            def attn_span(b, q0, qw, hold_proj=0, after_tail=()):
                bt0 = offs[b] // P
                qg = offs[b] + q0
                njs = (q0 + qw) // P
                G = max(1, SPAN // qw)   # k-tiles packed per S-tile
                HS = 512  # per-head half stride: the two heads' concurrent
                #           matmul accumulation groups must be in separate
                #           PSUM banks (same-bank concurrent groups fault)
                pts = []                 # per j: (pt tile, h0 col base, off)
                for jg in range(0, njs, G):
                    jn = min(G, njs - jg)
                    st = ps_st.tile([P, 1024], F32, tag="st", name="st")
                    pt = ptp.tile([P, 1024], BF16, tag="pt", name="pt")
                    for jj in range(jn):
                        j = jg + jj
                        off = max(0, j * P - q0)
                        moff = off if G == 1 else 0  # G>1: full width so the
                        #           packed EXP never reads unwritten PSUM
                        kb = offs[b] + j * P
                        cb = jj * qw
                        pts.append((pt, cb, off))
                        for h in range(HPC):
                            hb = h * DH
                            nc.tensor.matmul(
                                st[:, h * HS + cb + moff:h * HS + cb + qw],
                                kT_sb[hb:hb + DH, kb:kb + P],
                                qT_sb[hb:hb + DH, qg + moff:qg + qw],
                                start=True, stop=True)
                    eb = pts[jg][2] if G == 1 else 0
                    if eb == 0 and jn * qw == HS:
                        nc.scalar.activation(
                            pt[:, 0:2 * HS], st[:, 0:2 * HS],
                            AF.Exp, bias=eshift_sb[:])
                    else:
                        for h in range(HPC):
                            nc.scalar.activation(
                                pt[:, h * HS + eb:h * HS + jn * qw],
                                st[:, h * HS + eb:h * HS + jn * qw],
                                AF.Exp, bias=eshift_sb[:])
                    for jj in range(jn):
                        j = jg + jj
                        off = max(0, j * P - q0)
                        if j * P >= q0:  # diagonal block: keep q >= k
                            cb = jj * qw
                            for h in range(HPC):
                                dsl = slice(h * HS + cb + off,
                                            h * HS + cb + off + P)
                                nc.vector.tensor_tensor(
                                    pt[:, dsl], pt[:, dsl], ut_sb[:], OP.mult)
                        drain_inject(2)

                # phase B (deferred): PV accumulation, normalization tail and
                # c_proj tiles interleave with the NEXT span's phase A
                state = {}

                def emit_pv(j):
                    if j == 0:
                        # per-head accumulators, one full PSUM bank each so
                        # the two heads' accumulation groups never share a
                        # bank (required for qw < 512)
                        state["pv"] = [
                            ps_pv.tile([DH + 1, 512], F32, tag=f"pv{h}",
                                       name=f"pv{h}_{b}_{q0}")
                            for h in range(HPC)]
                    pt, cb, off = pts[j]
                    for h in range(HPC):
                        vc0 = h * (DH + 1)
                        nc.tensor.matmul(
                            state["pv"][h][:, off:qw],
                            v_nat[:, bt0 + j, vc0:vc0 + DH + 1],
                            pt[:, h * 512 + cb + off:h * 512 + cb + qw],
                            start=(j == 0), stop=(j == njs - 1))

                def emit_tail():
                    for h in range(HPC):
                        pvh = state["pv"][h]
                        hb = h * DH
                        den = rows.tile([1, qw], F32, tag="den")
                        nc.vector.tensor_copy(out=den, in_=pvh[DH:DH + 1, 0:qw])
                        rq = rows.tile([1, qw], F32, tag="rq")
                        nc.vector.reciprocal_approx_fast(out=rq, in_=den)
                        bc_sb = rows.tile([DH, qw], F32, tag="bcs")
                        nc.gpsimd.partition_broadcast(bc_sb[:], rq[:])
                        nc.vector.tensor_tensor(
                            yT_sb[hb:hb + DH, qg:qg + qw],
                            pvh[0:DH, 0:qw],
                            bc_sb[:], OP.mult)

                for j in range(0, njs, 2):
                    if j + 1 < njs:
                        inject_q.append(
                            (None, lambda j=j: (emit_pv(j), emit_pv(j + 1))))
                    else:
                        inject_q.append((None, lambda j=j: emit_pv(j)))
                inject_q.append((None, emit_tail))
                for f in after_tail:
                    inject_q.append((None, f))
                tts = list(range(qg // P, (qg + qw) // P))
                for tt in tts[:hold_proj]:
                    holdback.append(functools.partial(emit_proj_tile, tt))
                for tt in tts[hold_proj:]:
                    inject_q.append(
                        (None, functools.partial(emit_proj_tile, tt)))

            # ---- main emission ----
            # Interleave batches (sort by q0) so each span's deferred
            # phase B drains into a following span with at least as many
            # k-tile slots; keep a big span last so the terminal drain is
            # dense PE work (HAM stays warm).
            order = sorted(spans, key=lambda s: (s[1], s[0]))
            if len(order) >= 2 and order[-1][2] < SPAN <= order[-2][2]:
                order[-1], order[-2] = order[-2], order[-1]
            last = len(order) - 1
            for i, (b, q0, qw) in enumerate(order):
                need_end = offs[b] + q0 + qw
                for g in range(NG):
                    if groups[g][0] < need_end:
                        require_group(g)
                # queue qkv filler for the next two spans' needs
                nneed = max((offs[order[j][0]] + order[j][1] + order[j][2]
                             for j in range(i + 1, min(i + 3, len(order)))),
                            default=0)
                for g in range(NG):
                    if groups[g][0] < nneed:
                        queue_group(g)
                if i == last and holdback:
                    # half the held-back proj tiles pad this span's phase A;
                    # the rest keep the PE hot through the final tail chain
                    mid = len(holdback) // 2
                    for f in holdback[:mid]:
                        inject_q.append((None, f))
                    attn_span(b, q0, qw, after_tail=holdback[mid:])
                else:
                    attn_span(b, q0, qw,
                              hold_proj=4 if i < 2 and i != last else 0)
            while inject_q:
                _pop_emit()

    nc.finalize()
    return nc


@functools.lru_cache(maxsize=4)
def _built(TP0, TP1):
    return build(TP0, TP1)


def _plan(attention_mask):
    m = np.asarray(attention_mask) != 0
    idx = [np.nonzero(m[b])[0] for b in range(B)]
    nv = [len(i) for i in idx]
    TPs = tuple(max(P, -(-n // P) * P) for n in nv)
    return TPs, idx, nv


def _prep_core(c, W_attn, b_attn, W_proj):
    bf = ml_dtypes.bfloat16
    q0 = c * HPC * DH
    qs = slice(q0, q0 + P)
    ks = slice(D + q0, D + q0 + P)
    vs = slice(2 * D + q0, 2 * D + q0 + P)
    wsl = np.concatenate(
        [W_attn[:, qs], W_attn[:, ks], W_attn[:, vs]], axis=1)  # [1024, 384]
    bq = b_attn[qs] * QSCALE
    zv = np.zeros(P, dtype=np.float32)  # v bias handled host-side
    return {
        "wqkv": np.ascontiguousarray(wsl.reshape(KC, P, 3 * P)).astype(bf),
        "bqkv": np.ascontiguousarray(
            np.stack([bq, b_attn[ks], zv], axis=1)).astype(np.float32),
        "wproj": np.ascontiguousarray(W_proj[qs, :]).astype(bf),
    }


def prepare(x, attention_mask, W_attn, b_attn, W_proj):
    bf = ml_dtypes.bfloat16
    x = np.asarray(x, dtype=np.float32)
    W_attn = np.asarray(W_attn, dtype=np.float32)
    b_attn = np.asarray(b_attn, dtype=np.float32)
    W_proj = np.asarray(W_proj, dtype=np.float32)

    TPs, idx, nv = _plan(attention_mask)
    offs = (0, TPs[0])
    TT = TPs[0] + TPs[1]
    NT = TT // P

    xc = np.zeros((TT, D), dtype=np.float32)
    for b in range(B):
        xc[offs[b]:offs[b] + nv[b]] = x[b][idx[b]]
    xT = np.ascontiguousarray(xc.T).astype(bf)

    mdv = np.full(TT, 1e-18, dtype=np.float32)
    for b in range(B):
        mdv[offs[b]:offs[b] + nv[b]] = 1.0
    mden = np.ascontiguousarray(
        mdv.reshape(NT, P).T[:, :, None]).astype(np.float32)

    in_maps = []
    for c in range(NCORES):
        m = _prep_core(c, W_attn, b_attn, W_proj)
        m["xT"] = xT
        m["mcolden"] = mden
        in_maps.append(m)
    return TPs, idx, nv, in_maps


def kernel(x, attention_mask, W_attn, b_attn, W_proj, b_proj):
    b_proj = np.asarray(b_proj, dtype=np.float32)
    b_attn = np.asarray(b_attn, dtype=np.float32)
    W_proj_f = np.asarray(W_proj, dtype=np.float32)
    TPs, idx, nv, in_maps = prepare(x, attention_mask, W_attn, b_attn, W_proj)
    offs = (0, TPs[0])
    nc = _built(*TPs)
    res = run_bass_kernel_spmd(nc, in_maps, core_ids=list(range(NCORES)))
    TT = TPs[0] + TPs[1]
    acc = np.zeros((TT, D), dtype=np.float32)
    for c in range(NCORES):
        acc += res.results[c]["out"].astype(np.float32)
    # v-bias correction: device used bias-free v; attention weights sum to 1
    # over valid keys, so valid rows need + b_v @ W_proj.
    bv = b_attn[2 * D:3 * D]
    corr = bv @ W_proj_f + b_proj  # [1024]
    y = np.broadcast_to(b_proj, (B, T, D)).copy()
    for b in range(B):
        y[b, idx[b]] = acc[offs[b]:offs[b] + nv[b]] + corr
    return y


# revision 18
# speedup vs baseline: 1.1131x; 1.1131x over previous
"""Trainium2 Bass kernel for CausalSelfAttention (B=2, T=2048, D=1024, H=16).

v5: mask-compacted sequences. ~half the tokens have attention_mask == 0;
masked keys provably don't affect valid queries (softmax over -inf) and
masked-query rows output exactly b_proj. So the host compacts each batch
to its valid tokens (order preserved -> causal structure is exactly
preserved), pads per batch to a multiple of 128, and the device runs the
same Megatron-style 8-core kernel on the short sequences (~1024+1152
tokens instead of 2048+2048). Attention area shrinks ~3.2x, projections
~1.9x. Valid-token rows are scattered back on the host; masked rows get
b_proj.

Device structure per core (heads {2c, 2c+1}), inherited from v4:
  - column-parallel c_attn (384 of 3072 features), full attention for
    2 heads x 2 batches, row-parallel c_proj; host sums 8 partials.
  - gapless-PE design (HAM): warm-up matmuls during initial DMA wait,
    QK pairs into disjoint PE row groups, phase-split spans with
    injectable filler (qkv chunks / deferred PV / c_proj tiles).
  - denominator trick: v_nat carries a mask column (1 valid / 1e-18
    pad) so the softmax denominator comes out of the PV matmul as
    row 64; normalization multiplies by its reciprocal broadcast.
  - pad-token rows produce small garbage and are discarded host-side,
    so the old query-mask multiply in the c_proj eviction is gone;
    evictions alternate DVE/ACT to balance engine load.
  - spans of <=512 queries; narrower remainder spans pack G=512//qw
    k-tiles into one [128,1024] S-tile so EXP stays wide.
"""

import functools
from collections import deque

import numpy as np
import ml_dtypes

import concourse.bass as bass
import concourse.mybir as mybir
import concourse.tile as tile
from concourse import bacc
from concourse.bass_utils import run_bass_kernel_spmd
from concourse.masks import make_upper_triangular, make_identity

BF16 = mybir.dt.bfloat16
F32 = mybir.dt.float32
AF = mybir.ActivationFunctionType
OP = mybir.AluOpType

B, T, D, NH = 2, 2048, 1024, 16
DH = 64                  # head dim
HPC = 2                  # heads per core
NCORES = 8
P = 128
KC = D // P              # 8 contraction tiles for qkv
SPAN = 512               # max q-span per softmax pass
QSCALE = 1.0 / np.sqrt(DH)
ESHIFT = -10.0           # constant exp shift; cancels in softmax ratio
VW = 2 * DH + 2          # v_nat row width: [h0 v | m | h1 v | m]
GW = 512                 # qkv token-group width


def build(TP0, TP1):
    TPs = (TP0, TP1)
    offs = (0, TP0)
    TT = TP0 + TP1
    NT = TT // P

    # qkv token groups; first one small so its DMA lands just as the
    # PE warm-up stream ends
    groups = []
    t = 0
    while t < TT:
        w = min(256 if t == 0 else GW, TT - t)
        groups.append((t, w))
        t += w
    NG = len(groups)

    # attention spans (b, local q0, width)
    spans = []
    for b in (0, 1):
        q = 0
        while q < TPs[b]:
            w = min(SPAN, TPs[b] - q)
            spans.append((b, q, w))
            q += w

    nc = bacc.Bacc(None)

    xT = nc.dram_tensor("xT", [D, TT], BF16, kind="ExternalInput")
    wqkv = nc.dram_tensor("wqkv", [KC, P, 3 * P], BF16, kind="ExternalInput")
    bqkv = nc.dram_tensor("bqkv", [P, 3], F32, kind="ExternalInput")
    wproj = nc.dram_tensor("wproj", [P, D], BF16, kind="ExternalInput")
    mcolden = nc.dram_tensor("mcolden", [P, NT, 1], F32, kind="ExternalInput")
    out = nc.dram_tensor("out", [TT, D], BF16, kind="ExternalOutput")

    with tile.TileContext(nc) as tc:
        with (
            tc.tile_pool(name="singles", bufs=1) as singles,
            tc.tile_pool(name="stage", bufs=2) as stage,
            tc.tile_pool(name="pt", bufs=24) as ptp,
            tc.tile_pool(name="rows", bufs=2) as rows,
            tc.tile_pool(name="outs", bufs=3) as outs,
            tc.tile_pool(name="st", bufs=2, space="PSUM") as ps_st,
            tc.tile_pool(name="pv", bufs=1, space="PSUM") as ps_pv,
            tc.tile_pool(name="popq", bufs=2, space="PSUM") as ps_pq,
        ):
            # warm-up fodder first: a DVE memset is ready ~0.3us in
            warm_in = singles.tile([P, P], BF16)
            nc.vector.memset(warm_in, 0.5)

            eshift_sb = singles.tile([P, 1], F32)
            nc.vector.memset(eshift_sb, ESHIFT)
            ut_sb = singles.tile([P, P], BF16)  # keep q >= k
            make_upper_triangular(nc, ut_sb, val=1.0, diag=True)
            ident = singles.tile([P, P], BF16)
            make_identity(nc, ident)

            # input DMAs, most-urgent first
            wqkv_sb = singles.tile([P, KC, 3 * P], BF16)
            nc.sync.dma_start(out=wqkv_sb, in_=wqkv.rearrange("k p m -> p k m"))
            xT_sb = singles.tile([P, KC, TT], BF16)
            g0w = groups[0][1]
            for k in range(KC):
                nc.sync.dma_start(out=xT_sb[:, k, 0:g0w],
                                  in_=xT[k * P:(k + 1) * P, 0:g0w])
            bqkv_sb = singles.tile([P, 3], F32)
            nc.sync.dma_start(out=bqkv_sb, in_=bqkv[:, :])
            mden_sb = singles.tile([P, NT, 1], F32)
            nc.sync.dma_start(out=mden_sb, in_=mcolden[:, :, :])
            wproj_sb = singles.tile([P, D], BF16)
            nc.sync.dma_start(out=wproj_sb, in_=wproj[:, :])

            # PE warm-up through the DMA wait. Wide (N=512) matmuls give
            # ~full streaming duty so the HAM un-throttles after ~3.4us;
            # narrow low-duty warm-ups never trip it and the real stream
            # then starts (and stays) at half clock.
            warm_rhs = singles.tile([P, 512], BF16)
            nc.vector.memset(warm_rhs, 0.25)
            for w in range(8):
                wps = ps_pq.tile([P, 512], F32, tag="popq", name=f"warm{w}")
                nc.tensor.matmul(wps[:], warm_in[:], warm_rhs[:],
                                 start=True, stop=True)

            qT_sb = singles.tile([P, TT], BF16)   # rows: h0 d0..63 | h1 d0..63
            kT_sb = singles.tile([P, TT], BF16)
            yT_sb = singles.tile([P, TT], BF16)
            v_nat = singles.tile([P, NT, VW], BF16)

            # denominator-mask columns for all k-tiles in two bulk ops
            nc.vector.tensor_copy(out=v_nat[:, :, DH:DH + 1], in_=mden_sb)
            nc.vector.tensor_copy(out=v_nat[:, :, VW - 1:VW], in_=mden_sb)

            # ---- injectable chunk emitters ----
            def emit_qkv_chunk(g, m):
                tok0, w = groups[g]
                if m == 0 and g + 1 < NG:   # prefetch next group's xT
                    nt0, nw = groups[g + 1]
                    for k in range(KC):
                        nc.sync.dma_start(
                            out=xT_sb[:, k, nt0:nt0 + nw],
                            in_=xT[k * P:(k + 1) * P, nt0:nt0 + nw])
                pq = ps_pq.tile([P, w], F32, tag="popq", name=f"pq{g}_{m}")
                for k in range(KC):
                    nc.tensor.matmul(
                        pq[:], wqkv_sb[:, k, m * P:(m + 1) * P],
                        xT_sb[:, k, tok0:tok0 + w],
                        start=(k == 0), stop=(k == KC - 1))
                csl = slice(tok0, tok0 + w)
                if m == 0:
                    nc.vector.tensor_scalar(
                        qT_sb[:, csl], pq[:], QSCALE, bqkv_sb[:, 0:1],
                        OP.mult, OP.add)
                elif m == 1:
                    nc.vector.tensor_scalar(
                        kT_sb[:, csl], pq[:], 1.0, bqkv_sb[:, 1:2],
                        OP.mult, OP.add)
                else:
                    vst = stage.tile([P, w], BF16, tag="vst", name=f"vst{g}")
                    nc.vector.tensor_copy(out=vst[:], in_=pq[:])
                    nq = w // P
                    j0 = tok0 // P
                    vtp = ps_pq.tile([P, nq, P], BF16, tag="popq",
                                     name=f"vtp{g}")
                    for q4 in range(nq):
                        nc.tensor.transpose(
                            vtp[:, q4, :], vst[:, q4 * P:(q4 + 1) * P],
                            ident[:])
                    nc.vector.tensor_copy(
                        out=v_nat[:, j0:j0 + nq, 0:DH], in_=vtp[:, :, 0:DH])
                    nc.vector.tensor_copy(
                        out=v_nat[:, j0:j0 + nq, DH + 1:2 * DH + 1],
                        in_=vtp[:, :, DH:2 * DH])

            evict_rr = [0]

            def emit_proj_tile(tt, endgame=False):
                # endgame tiles borrow the (dead) st pool for the second po
                # half: 4-deep eviction pipeline so back-to-back proj tiles
                # don't serialize on the 2-buf popq ring
                ob = outs.tile([P, D], BF16, tag="ob", name=f"ob{tt}")
                for half in range(2):
                    if endgame and half == 1:
                        pot = ps_st.tile([P, 1024], F32, tag="st",
                                         name=f"post{tt}")
                        po = pot[:, 0:512]
                    else:
                        pot = ps_pq.tile([P, 512], F32, tag="popq",
                                         name=f"po{tt}_{half}")
                        po = pot[:]
                    nc.tensor.matmul(
                        po, yT_sb[:, tt * P:(tt + 1) * P],
                        wproj_sb[:, half * 512:(half + 1) * 512],
                        start=True, stop=True)
                    osl = ob[:, half * 512:(half + 1) * 512]
                    if evict_rr[0] % 2 == 0:
                        nc.vector.tensor_copy(out=osl, in_=po)
                    else:
                        nc.scalar.copy(out=osl, in_=po)
                    evict_rr[0] += 1
                nc.sync.dma_start(out=out[tt * P:(tt + 1) * P, :], in_=ob)

            # pending injectable work, popped between attention k-tiles
            inject_q = deque()
            pending_qkv = {}

            def _pop_emit():
                tag, fn = inject_q.popleft()
                if tag is not None:
                    pending_qkv[tag] -= 1
                fn()

            # paced draining: spread the filler queue evenly over the
            # remaining k-tile drain points so late (PE-starved) phases
            # aren't left bare by greedy early draining
            steps_left = [sum((q0 + qw) // P for _, q0, qw in spans)]

            def drain_paced():
                left = max(1, steps_left[0])
                steps_left[0] -= 1
                n = min(4, -(-len(inject_q) // left))
                for _ in range(n):
                    if not inject_q:
                        return
                    _pop_emit()

            qkv_done = set()

            def require_group(g):
                if g in qkv_done:
                    while pending_qkv.get(g, 0) > 0:
                        _pop_emit()
                    return
                qkv_done.add(g)
                for m in range(3):
                    emit_qkv_chunk(g, m)

            def queue_group(g):
                if g in qkv_done:
                    return
                qkv_done.add(g)
                pending_qkv[g] = 3
                for m in range(3):
                    inject_q.append(
                        (g, functools.partial(emit_qkv_chunk, g, m)))

            holdback = []  # early proj tiles deferred to the endgame

            def attn_span(b, q0, qw, hold_proj=0, endgame_extras=None):
                bt0 = offs[b] // P
                qg = offs[b] + q0
                njs = (q0 + qw) // P
                G = max(1, SPAN // qw)   # k-tiles packed per S-tile
                HS = 512  # per-head half stride: the two heads' concurrent
                #           matmul accumulation groups must be in separate
                #           PSUM banks (same-bank concurrent groups fault)
                pts = []                 # per j: (pt tile, h0 col base, off)
                for jg in range(0, njs, G):
                    jn = min(G, njs - jg)
                    st = ps_st.tile([P, 1024], F32, tag="st", name="st")
                    pt = ptp.tile([P, 1024], BF16, tag="pt", name="pt")
                    for jj in range(jn):
                        j = jg + jj
                        off = max(0, j * P - q0)
                        moff = off if G == 1 else 0  # G>1: full width so the
                        #           packed EXP never reads unwritten PSUM
                        kb = offs[b] + j * P
                        cb = jj * qw
                        pts.append((pt, cb, off))
                        for h in range(HPC):
                            hb = h * DH
                            nc.tensor.matmul(
                                st[:, h * HS + cb + moff:h * HS + cb + qw],
                                kT_sb[hb:hb + DH, kb:kb + P],
                                qT_sb[hb:hb + DH, qg + moff:qg + qw],
                                start=True, stop=True)
                    eb = pts[jg][2] if G == 1 else 0
                    if eb == 0 and jn * qw == HS:
                        nc.scalar.activation(
                            pt[:, 0:2 * HS], st[:, 0:2 * HS],
                            AF.Exp, bias=eshift_sb[:])
                    else:
                        for h in range(HPC):
                            nc.scalar.activation(
                                pt[:, h * HS + eb:h * HS + jn * qw],
                                st[:, h * HS + eb:h * HS + jn * qw],
                                AF.Exp, bias=eshift_sb[:])
                    for jj in range(jn):
                        j = jg + jj
                        off = max(0, j * P - q0)
                        if j * P >= q0:  # diagonal block: keep q >= k
                            cb = jj * qw
                            for h in range(HPC):
                                dsl = slice(h * HS + cb + off,
                                            h * HS + cb + off + P)
                                nc.vector.tensor_tensor(
                                    pt[:, dsl], pt[:, dsl], ut_sb[:], OP.mult)
                        drain_paced()

                # phase B (deferred): PV accumulation, normalization tail and
                # c_proj tiles interleave with the NEXT span's phase A
                state = {}

                def emit_pv(j):
                    if j == 0:
                        # per-head accumulators, one full PSUM bank each so
                        # the two heads' accumulation groups never share a
                        # bank (required for qw < 512)
                        state["pv"] = [
                            ps_pv.tile([DH + 1, 512], F32, tag=f"pv{h}",
                                       name=f"pv{h}_{b}_{q0}")
                            for h in range(HPC)]
                    pt, cb, off = pts[j]
                    for h in range(HPC):
                        vc0 = h * (DH + 1)
                        nc.tensor.matmul(
                            state["pv"][h][:, off:qw],
                            v_nat[:, bt0 + j, vc0:vc0 + DH + 1],
                            pt[:, h * 512 + cb + off:h * 512 + cb + qw],
                            start=(j == 0), stop=(j == njs - 1))

                def emit_tail():
                    for h in range(HPC):
                        pvh = state["pv"][h]
                        hb = h * DH
                        den = rows.tile([1, qw], F32, tag="den")
                        nc.vector.tensor_copy(out=den, in_=pvh[DH:DH + 1, 0:qw])
                        rq = rows.tile([1, qw], F32, tag="rq")
                        nc.vector.reciprocal_approx_fast(out=rq, in_=den)
                        bc_sb = rows.tile([DH, qw], F32, tag="bcs")
                        nc.gpsimd.partition_broadcast(bc_sb[:], rq[:])
                        nc.vector.tensor_tensor(
                            yT_sb[hb:hb + DH, qg:qg + qw],
                            pvh[0:DH, 0:qw],
                            bc_sb[:], OP.mult)

                endg = endgame_extras is not None
                ivl = deque(endgame_extras or ())
                for j in range(0, njs, 2):
                    if j + 1 < njs:
                        inject_q.append(
                            (None, lambda j=j: (emit_pv(j), emit_pv(j + 1))))
                    else:
                        inject_q.append((None, lambda j=j: emit_pv(j)))
                    if ivl:  # keep the PE hot between the final PV bursts
                        inject_q.append((None, ivl.popleft()))
                inject_q.append((None, emit_tail))
                while ivl:
                    inject_q.append((None, ivl.popleft()))
                tts = list(range(qg // P, (qg + qw) // P))
                for tt in tts[:hold_proj]:
                    holdback.append(
                        functools.partial(emit_proj_tile, tt, True))
                for tt in tts[hold_proj:]:
                    inject_q.append(
                        (None, functools.partial(emit_proj_tile, tt, endg)))

            # ---- main emission ----
            # Interleave batches (sort by q0) so each span's deferred
            # phase B drains into a following span with at least as many
            # k-tile slots; keep a big span last so the terminal drain is
            # dense PE work (HAM stays warm).
            order = sorted(spans, key=lambda s: (s[1], s[0]))
            if len(order) >= 2 and order[-1][2] < SPAN <= order[-2][2]:
                order[-1], order[-2] = order[-2], order[-1]
            last = len(order) - 1
            for i, (b, q0, qw) in enumerate(order):
                need_end = offs[b] + q0 + qw
                for g in range(NG):
                    if groups[g][0] < need_end:
                        require_group(g)
                # queue qkv filler for the next two spans' needs
                nneed = max((offs[order[j][0]] + order[j][1] + order[j][2]
                             for j in range(i + 1, min(i + 3, len(order)))),
                            default=0)
                for g in range(NG):
                    if groups[g][0] < nneed:
                        queue_group(g)
                if i == last:
                    attn_span(b, q0, qw, endgame_extras=holdback)
                else:
                    attn_span(b, q0, qw, hold_proj=4 if i < 2 else 0)
            while inject_q:
                _pop_emit()

    nc.finalize()
    return nc


@functools.lru_cache(maxsize=4)
def _built(TP0, TP1):
    return build(TP0, TP1)


def _plan(attention_mask):
    m = np.asarray(attention_mask) != 0
    idx = [np.nonzero(m[b])[0] for b in range(B)]
    nv = [len(i) for i in idx]
    TPs = tuple(max(P, -(-n // P) * P) for n in nv)
    return TPs, idx, nv


def _prep_core(c, W_attn, b_attn, W_proj):
    bf = ml_dtypes.bfloat16
    q0 = c * HPC * DH
    qs = slice(q0, q0 + P)
    ks = slice(D + q0, D + q0 + P)
    vs = slice(2 * D + q0, 2 * D + q0 + P)
    wsl = np.concatenate(
        [W_attn[:, qs], W_attn[:, ks], W_attn[:, vs]], axis=1)  # [1024, 384]
    bq = b_attn[qs] * QSCALE
    zv = np.zeros(P, dtype=np.float32)  # v bias handled host-side
    return {
        "wqkv": np.ascontiguousarray(wsl.reshape(KC, P, 3 * P)).astype(bf),
        "bqkv": np.ascontiguousarray(
            np.stack([bq, b_attn[ks], zv], axis=1)).astype(np.float32),
        "wproj": np.ascontiguousarray(W_proj[qs, :]).astype(bf),
    }


def prepare(x, attention_mask, W_attn, b_attn, W_proj):
    bf = ml_dtypes.bfloat16
    x = np.asarray(x, dtype=np.float32)
    W_attn = np.asarray(W_attn, dtype=np.float32)
    b_attn = np.asarray(b_attn, dtype=np.float32)
    W_proj = np.asarray(W_proj, dtype=np.float32)

    TPs, idx, nv = _plan(attention_mask)
    offs = (0, TPs[0])
    TT = TPs[0] + TPs[1]
    NT = TT // P

    xc = np.zeros((TT, D), dtype=np.float32)
    for b in range(B):
        xc[offs[b]:offs[b] + nv[b]] = x[b][idx[b]]
    xT = np.ascontiguousarray(xc.T).astype(bf)

    mdv = np.full(TT, 1e-18, dtype=np.float32)
    for b in range(B):
        mdv[offs[b]:offs[b] + nv[b]] = 1.0
    mden = np.ascontiguousarray(
        mdv.reshape(NT, P).T[:, :, None]).astype(np.float32)

    in_maps = []
    for c in range(NCORES):
        m = _prep_core(c, W_attn, b_attn, W_proj)
        m["xT"] = xT
        m["mcolden"] = mden
        in_maps.append(m)
    return TPs, idx, nv, in_maps


def kernel(x, attention_mask, W_attn, b_attn, W_proj, b_proj):
    b_proj = np.asarray(b_proj, dtype=np.float32)
    b_attn = np.asarray(b_attn, dtype=np.float32)
    W_proj_f = np.asarray(W_proj, dtype=np.float32)
    TPs, idx, nv, in_maps = prepare(x, attention_mask, W_attn, b_attn, W_proj)
    offs = (0, TPs[0])
    nc = _built(*TPs)
    res = run_bass_kernel_spmd(nc, in_maps, core_ids=list(range(NCORES)))
    TT = TPs[0] + TPs[1]
    acc = np.zeros((TT, D), dtype=np.float32)
    for c in range(NCORES):
        acc += res.results[c]["out"].astype(np.float32)
    # v-bias correction: device used bias-free v; attention weights sum to 1
    # over valid keys, so valid rows need + b_v @ W_proj.
    bv = b_attn[2 * D:3 * D]
    corr = bv @ W_proj_f + b_proj  # [1024]
    y = np.broadcast_to(b_proj, (B, T, D)).copy()
    for b in range(B):
        y[b, idx[b]] = acc[offs[b]:offs[b] + nv[b]] + corr
    return y
